# revision 1
# baseline (speedup 1.0000x reference)
"""DCNv2 (modulated deformable conv) Trainium2 kernel.

8 cores = 4 batch samples x 2 image halves. Per core:
  1. Offset conv on PE (fp32): om[27, 8192].
  2. Hat planes (DVE/ACT): A[d][tap,pix] = relu(1-|dy-(d-2)|)*sigmoid(m),
     t[tap,pix136] = dx + 3 + kx + j. Round-trip DRAM for affine replication.
  3. Per row: M[125, 1224] = relu(1 - |x_l - t|) * A  (partition = x_l*5+d).
  4. Stage-1 matmuls: host-prebuilt x-window stationaries [125, 64] x M
     -> v[64, (k, 17w+j)] = exact bilinear samples * mask (|offset|<=2).
  5. Stage-2: out[o,.] += W_k.T @ v_k over 9 taps; bias on evac.
"""
import sys
sys.path.insert(0, "/opt/trn_rl_repo")
import numpy as np
import concourse.bass as bass
import concourse.tile as tile
from concourse import bacc, mybir
from concourse.bass_utils import run_bass_kernel_spmd

F32, BF16 = mybir.dt.float32, mybir.dt.bfloat16
MUL, ADD = mybir.AluOpType.mult, mybir.AluOpType.add

B, C, O, H, W = 4, 64, 128, 128, 128
K = 9
HH = 64
P = HH * W
CW, NW, XW, ND = 17, 8, 25, 5
WJ = NW * CW           # 136
N1 = K * WJ            # 1224
RB = 4                 # rows per stage-2 block
NCORES = 8

_cache = {}


def _ap(base, dims):
    """Manual AP: keep base partition dim, replace free dims."""
    return bass.AP(base.tensor, base.offset, [base.ap[0]] + dims)


def build_bass(debug=False):
    nc = bacc.Bacc("TRN2", target_bir_lowering=False, debug=False,
                   num_devices=NCORES)
    dp = lambda n, s, dt, out=False: nc.dram_tensor(
        n, s, dt, kind="ExternalOutput" if out else "ExternalInput").ap()

    feat_d = dp("feat", [2 * C, 66 * 130], F32)
    xwin_d = dp("xwin", [128, 66 * NW * C], BF16)
    womr_d = dp("womr", [2 * C, 9 * 27], F32)
    w2r_d = dp("w2r", [C, K * O], BF16)
    bias_d = dp("bias", [O, 1], F32)
    bom_d = dp("bom", [27, 1], F32)
    kx3_d = dp("kx3", [36, 1], F32)
    xlb_d = dp("xlb", [125, 1], F32)
    dbias_d = dp("dbias", [36, 5], F32)
    zb125_d = dp("zb125", [125, 1], F32)
    zb125p1_d = dp("zb125p1", [125, 1], F32)
    out_d = dp("out", [O, P], F32, out=True)
    dbg = {}
    if debug:
        dbg["om"] = dp("dbg_om", [27, P], F32, out=True)
        dbg["M"] = dp("dbg_M", [125, N1], F32, out=True)
        dbg["v"] = dp("dbg_v", [C, N1], F32, out=True)

    omdram = nc.dram_tensor("omdram", [27, P], F32).ap()
    adram = nc.dram_tensor("adram", [ND * K * HH * WJ], BF16).ap()
    tdram = nc.dram_tensor("tdram", [K * HH * WJ], BF16).ap()
    a4 = adram.rearrange("(d k r c) -> d k r c", d=ND, k=K, r=HH)
    t3 = tdram.rearrange("(k r c) -> k r c", k=K, r=HH)

    with tile.TileContext(nc) as tc:
        with (
            tc.tile_pool(name="big", bufs=1) as big,
            tc.tile_pool(name="io", bufs=3) as io,
            tc.tile_pool(name="mpool", bufs=3) as mpool,
            tc.tile_pool(name="vpool", bufs=6) as vpool,
            tc.tile_pool(name="ps1", bufs=2, space="PSUM") as ps1,
            tc.tile_pool(name="ps2", bufs=4, space="PSUM") as ps2,
            tc.tile_pool(name="small", bufs=1) as small,
        ):
            womr = small.tile([2 * C, 9 * 27], F32)
            nc.sync.dma_start(out=womr[:], in_=womr_d[:])
            w2r = small.tile([C, K * O], BF16)
            nc.sync.dma_start(out=w2r[:], in_=w2r_d[:])
            bias = small.tile([O, 1], F32)
            nc.sync.dma_start(out=bias[:], in_=bias_d[:])
            bom = small.tile([27, 1], F32)
            nc.sync.dma_start(out=bom[:], in_=bom_d[:])
            kx3 = small.tile([36, 1], F32)
            nc.sync.dma_start(out=kx3[:], in_=kx3_d[:])
            xlb = small.tile([125, 1], F32)
            nc.sync.dma_start(out=xlb[:], in_=xlb_d[:])
            dbias = small.tile([36, 5], F32)
            nc.sync.dma_start(out=dbias[:], in_=dbias_d[:])
            zb125 = small.tile([125, 1], F32)
            nc.sync.dma_start(out=zb125[:], in_=zb125_d[:])
            zb125p1 = small.tile([125, 1], F32)
            nc.sync.dma_start(out=zb125p1[:], in_=zb125p1_d[:])

            feat = big.tile([2 * C, 66 * 130], F32)
            nc.sync.dma_start(out=feat[:], in_=feat_d[:])

            # ---- 1. offset conv ----
            for ch in range(16):
                r0 = ch * 4
                pom = ps1.tile([27, 512], F32, tag="pom")
                for t9 in range(9):
                    dy, dx = t9 // 3, t9 % 3
                    base = feat[:, (r0 + dy) * 130 + dx:(r0 + dy) * 130 + dx + 1]
                    rhs = _ap(base, [[130, 4], [1, 128]])
                    nc.tensor.matmul(pom[:], womr[:, t9 * 27:(t9 + 1) * 27],
                                     rhs, start=(t9 == 0), stop=(t9 == 8))
                omc = io.tile([27, 512], F32, tag="omc")
                nc.vector.tensor_scalar_add(omc[:], pom[:], bom[:])
                nc.sync.dma_start(out=omdram[:, ch * 512:(ch + 1) * 512],
                                  in_=omc[:])
            if debug:
                nc.sync.dma_start(out=dbg["om"][:], in_=omdram[:])

            # ---- 2. fat planes [36, 2056] ----
            dyf = big.tile([36, 2056], F32)
            dxf = big.tile([36, 2056], F32)
            msf = big.tile([36, 2056], F32)
            for f in (dyf, dxf, msf):
                nc.vector.memset(f[:], 0.0)
            for q in range(4):
                pix = slice(q * 2048, (q + 1) * 2048)
                nc.sync.dma_start(out=dyf[q * 9:(q + 1) * 9, 0:2048],
                                  in_=omdram[0:18:2, pix])
                nc.sync.dma_start(out=dxf[q * 9:(q + 1) * 9, 0:2048],
                                  in_=omdram[1:18:2, pix])
                nc.sync.dma_start(out=msf[q * 9:(q + 1) * 9, 0:2048],
                                  in_=omdram[18:27, pix])
            nc.scalar.activation(msf[:, 0:2048], msf[:, 0:2048],
                                 mybir.ActivationFunctionType.Sigmoid,
                                 bias=dbias[:, 2:3])
            nc.vector.memset(msf[:, 2048:2056], 0.0)

            # t136 = dx (136-layout) -> tdram (bf16); integer part added later
            dxv = _ap(dxf[:], [[128, 16], [17, 8], [1, 17]])
            t136b = big.tile([36, 2176], BF16)
            nc.vector.tensor_copy(t136b[:].rearrange(
                "p (a b c) -> p a b c", a=16, b=8, c=17), dxv)
            for q in range(4):
                nc.sync.dma_start(
                    out=t3[:, q * 16:(q + 1) * 16, :],
                    in_=t136b[q * 9:(q + 1) * 9, :].rearrange(
                        "p (r c) -> p r c", r=16))

            # A planes -> adram (bf16)
            msv = _ap(msf[:], [[128, 16], [17, 8], [1, 17]])
            for d5 in range(ND):
                ab = big.tile([36, 2176], F32, tag="aplane")
                dyv = _ap(dyf[:], [[128, 16], [17, 8], [1, 17]])
                ab4 = ab[:].rearrange("p (a b c) -> p a b c", a=16, b=8, c=17)
                nc.scalar.activation(ab4, dyv,
                                     mybir.ActivationFunctionType.Abs,
                                     bias=dbias[:, d5:d5 + 1], scale=1.0)
                nc.vector.tensor_scalar(ab[:], ab[:], -1.0, 1.0,
                                        op0=MUL, op1=ADD)
                nc.vector.tensor_scalar_max(ab[:], ab[:], 0.0)
                abb = big.tile([36, 2176], BF16, tag="aplaneb")
                abb4 = abb[:].rearrange("p (a b c) -> p a b c", a=16, b=8, c=17)
                nc.vector.tensor_tensor(abb4, ab4, msv, op=MUL)
                for q in range(4):
                    nc.sync.dma_start(
                        out=a4[d5, :, q * 16:(q + 1) * 16, :],
                        in_=abb[q * 9:(q + 1) * 9, :].rearrange(
                            "p (r c) -> p r c", r=16))

            jkx = big.tile([125, N1], BF16)
            nc.gpsimd.iota(jkx[:], pattern=[[0, 3], [1, 3], [0, 8], [1, 17]],
                           base=3, channel_multiplier=0,
                           allow_small_or_imprecise_dtypes=True)

            # ---- 3-5. rows ----
            for blk in range(HH // RB):
                xwb = io.tile([128, 6 * NW * C], BF16, tag="xwb")
                nc.sync.dma_start(
                    out=xwb[:],
                    in_=xwin_d[:, blk * RB * NW * C:(blk * RB + 6) * NW * C])
                pouts = []
                vts = []
                for ri in range(RB):
                    row = blk * RB + ri
                    mt = mpool.tile([128, N1], BF16, tag="m")
                    nc.vector.memset(mt[96:128, :], 0.0)
                    texp = mpool.tile([125, N1], BF16, tag="texp")
                    nc.sync.dma_start(
                        out=texp[:].rearrange("p (k c) -> p k c", k=K),
                        in_=t3[:, row, :].unsqueeze(0)
                        .broadcast_to([125, K, WJ]))
                    for d5 in range(ND):
                        nc.sync.dma_start(
                            out=mt[d5 * 25:(d5 + 1) * 25, :].rearrange(
                                "p (k c) -> p k c", k=K),
                            in_=a4[d5, :, row, :].unsqueeze(0)
                            .broadcast_to([25, K, WJ]))
                    sab = mpool.tile([125, N1], BF16, tag="sab")
                    nc.vector.scalar_tensor_tensor(
                        sab[:], texp[:], xlb[:], jkx[:],
                        op0=mybir.AluOpType.subtract, op1=ADD)
                    nc.scalar.activation(sab[:], sab[:],
                                         mybir.ActivationFunctionType.Abs,
                                         bias=zb125[:])
                    nc.scalar.activation(sab[:], sab[:],
                                         mybir.ActivationFunctionType.Relu,
                                         bias=zb125p1[:], scale=-1.0)
                    nc.vector.tensor_tensor(mt[0:125, :], mt[0:125, :],
                                            sab[:], op=MUL)
                    if debug and row == 0:
                        mdb = mpool.tile([125, N1], F32, tag="mdb")
                        nc.vector.tensor_copy(mdb[:], mt[0:125, :])
                        nc.sync.dma_start(out=dbg["M"][:], in_=mdb[:])
                    # stage-1: per-ky stationaries (window base row+ky-1)
                    vt = vpool.tile([C, N1], BF16, tag="v")
                    vts.append(vt)
                    mt3 = mt[:].rearrange("p (k wj) -> p k wj", k=K)
                    for w8 in range(NW):
                        pv = ps1.tile([C, K * CW], F32, tag="pv")
                        for ky in range(3):
                            ti = (ri + ky) * NW + w8
                            nc.tensor.matmul(
                                pv[:, ky * 3 * CW:(ky + 1) * 3 * CW],
                                xwb[:, ti * C:(ti + 1) * C],
                                mt3[:, 3 * ky:3 * ky + 3,
                                    w8 * CW:(w8 + 1) * CW],
                                start=(ky == 0), stop=(ky == 2))
                        dst = _ap(vt[:, w8 * CW:w8 * CW + 1], [[WJ, K], [1, CW]])
                        nc.vector.tensor_copy(
                            dst, pv[:].rearrange("c (k j) -> c k j", k=K))
                    if debug and row == 0:
                        vdb = vpool.tile([C, N1], F32, tag="vdb")
                        nc.vector.tensor_copy(vdb[:], vt[:])
                        nc.sync.dma_start(out=dbg["v"][:], in_=vdb[:])
                    pt = ps2.tile([O, WJ], F32, tag="pout")
                    pouts.append(pt)
                for k9 in range(K):
                    for ri in range(RB):
                        nc.tensor.matmul(
                            pouts[ri][:], w2r[:, k9 * O:(k9 + 1) * O],
                            vts[ri][:, k9 * WJ:(k9 + 1) * WJ],
                            start=(k9 == 0), stop=(k9 == K - 1))
                osb = io.tile([O, RB * 128], F32, tag="osb")
                for ri in range(RB):
                    nc.vector.tensor_scalar_add(
                        osb[:, ri * 128:(ri + 1) * 128],
                        pouts[ri][:, 0:128], bias[:])
                nc.sync.dma_start(
                    out=out_d[:, blk * RB * 128:(blk + 1) * RB * 128],
                    in_=osb[:])
    nc.compile()
    return nc


def host_prep(input_feat, inter, weight, bias, w_om, b_om):
    import ml_dtypes
    maps = []
    womr = np.ascontiguousarray(
        w_om.transpose(1, 2, 3, 0).reshape(2 * C, 9 * 27)).astype(np.float32)
    w2r = np.ascontiguousarray(
        weight.reshape(O, C, K).transpose(1, 2, 0).reshape(C, K * O)
    ).astype(ml_dtypes.bfloat16)
    kx3 = np.tile((3.0 + np.arange(9) % 3).astype(np.float32), 4)[:, None]
    xlb = (np.arange(125) % 25).astype(np.float32)[:, None]
    for b in range(B):
        xpad = np.zeros((C, H + 6, 144), np.float32)
        xpad[:, 3:3 + H, 4:4 + W] = input_feat[b]
        featb = np.concatenate([input_feat[b], inter[b]], axis=0)
        for half in range(2):
            h0 = half * HH
            fs = np.zeros((2 * C, 66, 130), np.float32)
            r_lo, r_hi = max(0, h0 - 1), min(H, h0 + 65)
            fs[:, r_lo - (h0 - 1):r_hi - (h0 - 1), 1:129] = featb[:, r_lo:r_hi]
            xs = xpad[:, h0:h0 + 70, :]                    # [C, 70, 144]
            rows = (np.arange(66)[:, None, None, None]
                    + np.arange(ND)[None, None, None, :])  # r + d
            cols = (17 * np.arange(NW)[None, :, None, None]
                    + np.arange(XW)[None, None, :, None])
            xw = xs[:, rows, cols]                         # [C,66,NW,XW,ND]
            xw = xw.transpose(4, 3, 1, 2, 0).reshape(125, 66 * NW * C)
            xwp = np.zeros((128, 66 * NW * C), np.float32)
            xwp[:125] = xw
            maps.append({
                "feat": fs.reshape(2 * C, 66 * 130),
                "xwin": xwp.astype(ml_dtypes.bfloat16),
                "womr": womr, "w2r": w2r,
                "bias": np.asarray(bias, np.float32).reshape(O, 1),
                "bom": np.asarray(b_om, np.float32).reshape(27, 1),
                "kx3": kx3, "xlb": xlb,
                "dbias": np.tile(-(np.arange(5, dtype=np.float32) - 2),
                                 (36, 1)),
                "zb125": np.zeros((125, 1), np.float32),
                "zb125p1": np.ones((125, 1), np.float32),
            })
    return maps


def kernel(input_feat, inter, weight, bias, w_om, b_om):
    if "nc" not in _cache:
        _cache["nc"] = build_bass(debug=False)
    nc = _cache["nc"]
    maps = host_prep(np.asarray(input_feat, np.float32),
                     np.asarray(inter, np.float32),
                     np.asarray(weight, np.float32),
                     np.asarray(bias, np.float32),
                     np.asarray(w_om, np.float32),
                     np.asarray(b_om, np.float32))
    res = run_bass_kernel_spmd(nc, maps, list(range(NCORES)))
    out = np.zeros((B, O, H, W), np.float32)
    for ci in range(NCORES):
        b, half = ci // 2, ci % 2
        out[b, :, half * HH:(half + 1) * HH] = \
            res.results[ci]["out"].reshape(O, HH, W)
    return out



# revision 22
# speedup vs baseline: 2.0613x; 2.0613x over previous
"""DCNv2 (modulated deformable conv) Trainium2 kernel, v2.

8 cores = 4 batch samples x 2 image halves. Per core:
  1. Offset conv on PE (bf16 moving): om[27, 8192] -> omdram (bf16).
  2. Hat planes: A[d] = relu(1-|dy-(d-2)|)*sigmoid(m) on [36, 2176]
     tiles (ACT abs + DVE); t-plane = (dx + 3 + kx + j) in fp16,
     (k, w8, j) layout. Written to DRAM row-major so per-row
     broadcasts are single >=2KB-elem DMAs.
  3. Per row r: two broadcast DMAs (t-row -> 125 parts, A-row -> 25x
     replication), then M = relu((1-|t' - x_l|)*A):
     ACT Abs(texp, bias=-x_l) -> DVE stt (a-1)*A -> DVE relu-ts.
  4. Stage-1: 24 matmuls (125-contraction) into 2 psum supertiles
     [64, 1024] (bank-aligned 153-col regions); one strided Copy per
     supertile (ACT for half 0, DVE for half 1) into vt [64, 1224].
  5. Stage-2: per row 9 matmuls w2r[64,128].T @ vt-slices into
     row-triple psum [128, 3*136]; ACT adds bias on evac, bf16 out.
"""
import sys
sys.path.insert(0, "/opt/trn_rl_repo")
import numpy as np
import concourse.bass as bass
import concourse.tile as tile
from concourse import bacc, mybir
from concourse.bass_utils import run_bass_kernel_spmd

F32, BF16, FP16 = mybir.dt.float32, mybir.dt.bfloat16, mybir.dt.float16
ALU = mybir.AluOpType
AF = mybir.ActivationFunctionType

B, C, O, H, W = 4, 64, 128, 128, 128
K, HH = 9, 64
P = HH * W
CW, NW, ND, NP = 17, 8, 5, 125
WJ = NW * CW           # 136
N1 = K * WJ            # 1224
RG = HH + 2            # 66 xwin rowgroups
XSL = 16               # xwb ring slots
NCORES = 8

_cache = {}


def _ap(base, dims):
    """Manual AP: keep base partition dim, replace free dims."""
    return bass.AP(base.tensor, base.offset, [base.ap[0]] + dims)


def build_bass(debug=False):
    nc = bacc.Bacc("TRN2", target_bir_lowering=False, debug=False,
                   num_devices=NCORES)
    dp = lambda n, s, dt, out=False: nc.dram_tensor(
        n, s, dt, kind="ExternalOutput" if out else "ExternalInput").ap()
    dbg = {}
    if debug:
        dbg["om"] = dp("dbg_om", [27, P], BF16, out=True)
        dbg["t136"] = dp("dbg_t136", [36, 2176], FP16, out=True)
        dbg["abb"] = dp("dbg_abb", [ND * 36, 2176], BF16, out=True)
        dbg["mrow"] = dp("dbg_mrow", [NP, N1], BF16, out=True)
        dbg["vt"] = dp("dbg_vt", [C, N1], BF16, out=True)
        dbg["texp"] = dp("dbg_texp", [NP, N1], FP16, out=True)
        dbg["amt"] = dp("dbg_amt", [NP, N1], BF16, out=True)

    feat_d = dp("feat", [2 * C, RG * 130], BF16)
    xwin_d = dp("xwin", [NP, RG * NW * C], BF16)
    womr_d = dp("womr", [2 * C, 9 * 27], BF16)
    w2r_d = dp("w2r", [C, K * O], BF16)
    jkt_d = dp("jkt", [36, 2176], FP16)
    bias_d = dp("bias", [O, 1], F32)
    bom_d = dp("bom", [27, 1], F32)
    dbias_d = dp("dbias", [36, 5], F32)
    mxlb_d = dp("mxlb", [NP, 1], F32)
    one125_d = dp("one125", [NP, 1], F32)
    out_d = dp("out", [O, P], BF16, out=True)

    omdram = nc.dram_tensor("omdram", [27, P], BF16).ap()
    tdram = nc.dram_tensor("tdram", [HH * N1], FP16).ap()
    adram = nc.dram_tensor("adram", [HH * ND * N1], BF16).ap()
    t_wv = tdram.rearrange("(r k c) -> k r c", k=K, c=WJ)       # write view
    t_rv = tdram.rearrange("(r c) -> r c", c=N1)                # read view
    a_wv = adram.rearrange("(r d k c) -> d k r c", d=ND, k=K, c=WJ)
    a_rv = adram.rearrange("(r d c) -> r d c", d=ND, c=N1)

    with tile.TileContext(nc) as tc:
        with (
            tc.tile_pool(name="small", bufs=1) as small,
            tc.tile_pool(name="big", bufs=1) as big,
            tc.tile_pool(name="plp", bufs=2) as plp,
            tc.tile_pool(name="io", bufs=3) as io,
            tc.tile_pool(name="rowp", bufs=3) as rowp,
            tc.tile_pool(name="vpool", bufs=4) as vpool,
            tc.tile_pool(name="ps", bufs=3, space="PSUM") as ps,
            tc.tile_pool(name="ps2", bufs=2, space="PSUM") as ps2,
        ):
            womr = small.tile([2 * C, 9 * 27], BF16)
            nc.sync.dma_start(out=womr[:], in_=womr_d[:])
            w2r = small.tile([C, K * O], BF16)
            nc.sync.dma_start(out=w2r[:], in_=w2r_d[:])
            jkt = small.tile([36, 2176], FP16)
            nc.sync.dma_start(out=jkt[:], in_=jkt_d[:])
            bias = small.tile([O, 1], F32)
            nc.sync.dma_start(out=bias[:], in_=bias_d[:])
            bom = small.tile([27, 1], F32)
            nc.sync.dma_start(out=bom[:], in_=bom_d[:])
            dbias = small.tile([36, 5], F32)
            nc.sync.dma_start(out=dbias[:], in_=dbias_d[:])
            mxlb = small.tile([NP, 1], F32)
            nc.sync.dma_start(out=mxlb[:], in_=mxlb_d[:])
            one125 = small.tile([NP, 1], F32)
            nc.sync.dma_start(out=one125[:], in_=one125_d[:])

            feat = big.tile([2 * C, RG * 130], BF16)
            nc.sync.dma_start(out=feat[:], in_=feat_d[:])
            xwb = big.tile([NP, XSL * NW * C], BF16)
            nc.sync.dma_start(out=xwb[:, 0:10 * NW * C],
                              in_=xwin_d[:, 0:10 * NW * C])
            loaded = 10

            # ---- 1. offset conv (bf16 moving, DVE bias-evac) ----
            for ch in range(16):
                pom = ps.tile([C, 1024], F32, tag="sup")
                for t9 in range(9):
                    dy, dx = t9 // 3, t9 % 3
                    base = feat[:, (ch * 4 + dy) * 130 + dx:
                                (ch * 4 + dy) * 130 + dx + 1]
                    rhs = _ap(base, [[130, 4], [1, 128]])
                    nc.tensor.matmul(pom[0:27, 0:512],
                                     womr[:, t9 * 27:(t9 + 1) * 27],
                                     rhs, start=(t9 == 0), stop=(t9 == 8))
                omc = io.tile([27, 512], BF16, tag="omc")
                nc.vector.tensor_scalar_add(omc[:], pom[0:27, 0:512], bom[:])
                nc.sync.dma_start(out=omdram[:, ch * 512:(ch + 1) * 512],
                                  in_=omc[:])
                if debug:
                    nc.sync.dma_start(
                        out=dbg["om"][:, ch * 512:(ch + 1) * 512], in_=omc[:])

            # ---- 2. hat planes [36, 2056/2176] ----
            dyf = big.tile([36, 2056], BF16)
            dxf = big.tile([36, 2056], BF16)
            msf = big.tile([36, 2056], BF16)
            for f in (dyf, dxf, msf):
                nc.vector.memset(f[:, 2048:2056], 0.0)
            for q in range(4):
                pix = slice(q * 2048, (q + 1) * 2048)
                nc.sync.dma_start(out=dyf[q * 9:(q + 1) * 9, 0:2048],
                                  in_=omdram[0:18:2, pix])
                nc.sync.dma_start(out=dxf[q * 9:(q + 1) * 9, 0:2048],
                                  in_=omdram[1:18:2, pix])
                nc.sync.dma_start(out=msf[q * 9:(q + 1) * 9, 0:2048],
                                  in_=omdram[18:27, pix])
            nc.scalar.activation(msf[:, 0:2048], msf[:, 0:2048], AF.Sigmoid,
                                 bias=dbias[:, 2:3])

            dxv = _ap(dxf[:], [[128, 16], [17, 8], [1, 17]])
            dyv = _ap(dyf[:], [[128, 16], [17, 8], [1, 17]])
            msv = _ap(msf[:], [[128, 16], [17, 8], [1, 17]])
            t136 = big.tile([36, 2176], FP16)
            nc.vector.tensor_tensor(
                t136[:].rearrange("p (a b c) -> p a b c", a=16, b=8, c=17),
                dxv, jkt[:].rearrange("p (a b c) -> p a b c",
                                      a=16, b=8, c=17), op=ALU.add)
            for q in range(4):
                nc.sync.dma_start(
                    out=t_wv[:, q * 16:(q + 1) * 16, :],
                    in_=t136[q * 9:(q + 1) * 9, :].rearrange(
                        "p (r c) -> p r c", r=16))
            if debug:
                nc.sync.dma_start(out=dbg["t136"][:], in_=t136[:])

            for d5 in range(ND):
                ab = plp.tile([36, 2176], BF16, tag="aplane")
                ab4 = ab[:].rearrange("p (a b c) -> p a b c", a=16, b=8, c=17)
                nc.scalar.activation(ab4, dyv, AF.Abs,
                                     bias=dbias[:, d5:d5 + 1], scale=1.0)
                nc.vector.tensor_scalar(ab[:], ab[:], 1.0, None,
                                        op0=ALU.subtract)
                abb = plp.tile([36, 2176], BF16, tag="aplaneb")
                abb4 = abb[:].rearrange("p (a b c) -> p a b c",
                                        a=16, b=8, c=17)
                nc.vector.tensor_tensor(abb4, ab4, msv, op=ALU.mult)
                nc.vector.tensor_scalar(abb[:], abb[:], -1.0, 0.0,
                                        op0=ALU.mult, op1=ALU.max)
                for q in range(4):
                    nc.sync.dma_start(
                        out=a_wv[d5, :, q * 16:(q + 1) * 16, :],
                        in_=abb[q * 9:(q + 1) * 9, :].rearrange(
                            "p (r c) -> p r c", r=16))
                if debug:
                    nc.sync.dma_start(
                        out=dbg["abb"][d5 * 36:(d5 + 1) * 36, :], in_=abb[:])

            # ---- 3-5. rows ----
            pend = []
            tri = {}

            def stage2(r):
                bl = r % 8
                idx = bl % 3 if bl < 6 else bl - 6
                if idx == 0:
                    tri["p"] = ps2.tile([O, 3 * WJ], F32, tag="pout",
                                        name="pout")
                    tri["r0"] = r
                pout = tri["p"]
                vt = vts.pop(r)
                for k9 in range(K):
                    mbase = vt[:, k9 * CW:k9 * CW + 1]
                    mv = _ap(mbase, [[K * CW, NW], [1, CW]])
                    nc.tensor.matmul(pout[:, idx * WJ:(idx + 1) * WJ],
                                     w2r[:, k9 * O:(k9 + 1) * O],
                                     mv, start=(k9 == 0), stop=(k9 == K - 1))
                if bl in (2, 5, 7):
                    n = idx + 1
                    osb = io.tile([O, 3 * 128], BF16, tag="osb")
                    sbase = pout[:, 0:1]
                    src = _ap(sbase, [[WJ, n], [1, 128]])
                    nc.scalar.activation(osb[:, 0:n * 128], src, AF.Identity,
                                         bias=bias[:])
                    nc.sync.dma_start(
                        out=out_d[:, tri["r0"] * 128:(r + 1) * 128],
                        in_=osb[:, 0:n * 128])

            vts = {}
            for r in range(HH):
                if r % 8 == 0 and r > 0:
                    need = min(r + 9, RG - 1)
                    while loaded <= need:
                        s0 = loaded % XSL
                        cnt = min(XSL - s0, need - loaded + 1)
                        nc.sync.dma_start(
                            out=xwb[:, s0 * 512:(s0 + cnt) * 512],
                            in_=xwin_d[:, loaded * 512:(loaded + cnt) * 512])
                        loaded += cnt
                if r % 4 == 0:
                    texp4 = rowp.tile([NP, 4 * N1], FP16, tag="texp")
                    nc.sync.dma_start(
                        out=texp4[:],
                        in_=tdram[r * N1:(r + 4) * N1]
                        .unsqueeze(0).broadcast_to([NP, 4 * N1]))
                    amt4 = rowp.tile([NP, 4 * N1], BF16, tag="amt")
                    for d5 in range(ND):
                        nc.sync.dma_start(
                            out=amt4[d5 * 25:(d5 + 1) * 25, :].rearrange(
                                "p (w c) -> p w c", w=4),
                            in_=bass.AP(adram.tensor,
                                        adram.offset + (r * ND + d5) * N1,
                                        [[0, 25], [ND * N1, 4], [1, N1]]))
                rs = (r % 4) * N1
                texp = texp4[:, rs:rs + N1]
                amt = amt4[:, rs:rs + N1]
                sab = rowp.tile([NP, N1], BF16, tag="sab")
                nc.scalar.activation(sab[:], texp, AF.Abs,
                                     bias=mxlb[:], scale=1.0)
                nc.vector.scalar_tensor_tensor(sab[:], sab[:], one125[:],
                                               amt, op0=ALU.subtract,
                                               op1=ALU.mult)
                mrow = rowp.tile([NP, N1], BF16, tag="mrow")
                nc.vector.tensor_scalar(mrow[:], sab[:], -1.0, 0.0,
                                        op0=ALU.mult, op1=ALU.max)
                if debug and r == 5:
                    nc.sync.dma_start(out=dbg["mrow"][:], in_=mrow[:])
                    nc.sync.dma_start(out=dbg["texp"][:], in_=texp)
                    nc.sync.dma_start(out=dbg["amt"][:], in_=amt)
                vt = vpool.tile([C, N1], BF16, tag="v")
                vts[r] = vt
                for h2 in range(2):
                    sup = ps.tile([C, 1024], F32, tag="sup")
                    for g in range(4):
                        w8 = h2 * 4 + g
                        off = (g // 2) * 512 + (g % 2) * 153
                        for ky in range(3):
                            slot = (r + ky) % XSL
                            st = xwb[:, slot * 512 + w8 * C:
                                     slot * 512 + (w8 + 1) * C]
                            mbase = mrow[:, (3 * ky) * WJ + w8 * CW:
                                         (3 * ky) * WJ + w8 * CW + 1]
                            mv = _ap(mbase, [[WJ, 3], [1, CW]])
                            nc.tensor.matmul(
                                sup[:, off + 51 * ky:off + 51 * (ky + 1)],
                                st, mv, start=True, stop=True)
                    sbase = sup[:, 0:1]
                    src = _ap(sbase, [[512, 2], [153, 2], [1, 153]])
                    dst = vt[:, h2 * 612:(h2 + 1) * 612]
                    if h2 == 0:
                        nc.scalar.activation(dst, src, AF.Copy, bias=0.0)
                    else:
                        nc.vector.tensor_copy(dst, src)
                if debug and r == 5:
                    nc.sync.dma_start(out=dbg["vt"][:], in_=vt[:])
                pend.append(r)
                if r >= 1:
                    stage2(pend.pop(0))
            while pend:
                stage2(pend.pop(0))
    nc.compile()
    return nc


def host_prep(input_feat, inter, weight, bias, w_om, b_om):
    import ml_dtypes
    bf = ml_dtypes.bfloat16
    maps = []
    womr = np.ascontiguousarray(
        w_om.transpose(1, 2, 3, 0).reshape(2 * C, 9 * 27)).astype(bf)
    w2r = np.ascontiguousarray(
        weight.reshape(O, C, K).transpose(1, 2, 0).reshape(C, K * O)
    ).astype(bf)
    kk = (np.arange(K) % 3).astype(np.float32)
    jj = np.arange(CW, dtype=np.float32)
    jkt = np.broadcast_to(
        3.0 + kk[:, None, None] + jj[None, None, :],
        (K, NW, CW)).reshape(1, N1)                      # [1, (k, w8, j)]
    # jkt tile layout: [36=(q,k), (r16, w8, j)] -> value dep on (k, w8, j)
    jkt_t = np.zeros((36, 2176), np.float32)
    kwj = (3.0 + kk[:, None, None] + 0 * np.arange(NW)[None, :, None]
           + jj[None, None, :])                          # [9, 8, 17]
    for q in range(4):
        for k in range(K):
            jkt_t[q * 9 + k] = np.tile(kwj[k].reshape(1, NW * CW),
                                       (16, 1)).reshape(-1)
    dbias = np.tile(-(np.arange(5, dtype=np.float32) - 2), (36, 1))
    mxlb = -(np.arange(NP) % 25).astype(np.float32)[:, None]
    for b in range(B):
        xpad = np.zeros((C, H + 6, 144), np.float32)
        xpad[:, 3:3 + H, 4:4 + W] = input_feat[b]
        featb = np.concatenate([input_feat[b], inter[b]], axis=0)
        for half in range(2):
            h0 = half * HH
            fs = np.zeros((2 * C, RG, 130), np.float32)
            r_lo, r_hi = max(0, h0 - 1), min(H, h0 + 65)
            fs[:, r_lo - (h0 - 1):r_hi - (h0 - 1), 1:129] = featb[:, r_lo:r_hi]
            xs = xpad[:, h0:h0 + 70, :]                    # [C, 70, 144]
            rows = (np.arange(RG)[:, None, None, None]
                    + np.arange(ND)[None, None, None, :])  # r + d
            cols = (17 * np.arange(NW)[None, :, None, None]
                    + np.arange(25)[None, None, :, None])
            xw = xs[:, rows, cols]                         # [C,66,NW,25,ND]
            xw = xw.transpose(4, 3, 1, 2, 0).reshape(NP, RG * NW * C)
            maps.append({
                "feat": fs.reshape(2 * C, RG * 130).astype(bf),
                "xwin": xw.astype(bf),
                "womr": womr, "w2r": w2r,
                "jkt": jkt_t.astype(np.float16),
                "bias": np.asarray(bias, np.float32).reshape(O, 1),
                "bom": np.asarray(b_om, np.float32).reshape(27, 1),
                "dbias": dbias,
                "mxlb": mxlb,
                "one125": np.ones((NP, 1), np.float32),
            })
    return maps


def kernel(input_feat, inter, weight, bias, w_om, b_om):
    if "nc" not in _cache:
        _cache["nc"] = build_bass()
    nc = _cache["nc"]
    maps = host_prep(np.asarray(input_feat, np.float32),
                     np.asarray(inter, np.float32),
                     np.asarray(weight, np.float32),
                     np.asarray(bias, np.float32),
                     np.asarray(w_om, np.float32),
                     np.asarray(b_om, np.float32))
    res = run_bass_kernel_spmd(nc, maps, list(range(NCORES)))
    out = np.zeros((B, O, H, W), np.float32)
    for ci in range(NCORES):
        b, half = ci // 2, ci % 2
        out[b, :, half * HH:(half + 1) * HH] = \
            res.results[ci]["out"].astype(np.float32).reshape(O, HH, W)
    return out


# revision 25
# speedup vs baseline: 2.2140x; 1.0741x over previous
"""DCNv2 (modulated deformable conv) Trainium2 kernel, v2.

8 cores = 4 batch samples x 2 image halves. Per core:
  1. Offset conv on PE (bf16 moving): om[27, 8192] -> omdram (bf16).
  2. Hat planes: A[d] = relu(1-|dy-(d-2)|)*sigmoid(m) on [36, 2176]
     tiles (ACT abs + DVE); t-plane = (dx + 3 + kx + j) in fp16,
     (k, w8, j) layout. Written to DRAM row-major so per-row
     broadcasts are single >=2KB-elem DMAs.
  3. Per row r: two broadcast DMAs (t-row -> 125 parts, A-row -> 25x
     replication), then M = relu((1-|t' - x_l|)*A):
     ACT Abs(texp, bias=-x_l) -> DVE stt (a-1)*A -> DVE relu-ts.
  4. Stage-1: 24 matmuls (125-contraction) into 2 psum supertiles
     [64, 1024] (bank-aligned 153-col regions); one strided Copy per
     supertile (ACT for half 0, DVE for half 1) into vt [64, 1224].
  5. Stage-2: per row 9 matmuls w2r[64,128].T @ vt-slices into
     row-triple psum [128, 3*136]; ACT adds bias on evac, bf16 out.
"""
import sys
sys.path.insert(0, "/opt/trn_rl_repo")
import numpy as np
import concourse.bass as bass
import concourse.tile as tile
from concourse import bacc, mybir
from concourse.bass_utils import run_bass_kernel_spmd

F32, BF16, FP16 = mybir.dt.float32, mybir.dt.bfloat16, mybir.dt.float16
ALU = mybir.AluOpType
AF = mybir.ActivationFunctionType

B, C, O, H, W = 4, 64, 128, 128, 128
K, HH = 9, 64
P = HH * W
CW, NW, ND, NP = 17, 8, 5, 125
WJ = NW * CW           # 136
N1 = K * WJ            # 1224
RG = HH + 2            # 66 xwin rowgroups
XSL = 16               # xwb ring slots
NCORES = 8

_cache = {}


def _ap(base, dims):
    """Manual AP: keep base partition dim, replace free dims."""
    return bass.AP(base.tensor, base.offset, [base.ap[0]] + dims)


def build_bass(debug=False):
    nc = bacc.Bacc("TRN2", target_bir_lowering=False, debug=False,
                   num_devices=NCORES)
    dp = lambda n, s, dt, out=False: nc.dram_tensor(
        n, s, dt, kind="ExternalOutput" if out else "ExternalInput").ap()
    dbg = {}
    if debug:
        dbg["om"] = dp("dbg_om", [27, P], BF16, out=True)
        dbg["t136"] = dp("dbg_t136", [36, 2176], FP16, out=True)
        dbg["abb"] = dp("dbg_abb", [ND * 36, 2176], BF16, out=True)
        dbg["mrow"] = dp("dbg_mrow", [NP, N1], BF16, out=True)
        dbg["vt"] = dp("dbg_vt", [C, N1], BF16, out=True)
        dbg["texp"] = dp("dbg_texp", [NP, N1], FP16, out=True)
        dbg["amt"] = dp("dbg_amt", [NP, N1], BF16, out=True)

    feat_d = dp("feat", [2 * C, RG * 130], BF16)
    xwin_d = dp("xwin", [NP, RG * NW * C], BF16)
    womr_d = dp("womr", [2 * C, 9 * 27], BF16)
    w2r_d = dp("w2r", [C, K * O], BF16)
    jkt_d = dp("jkt", [36, 2176], FP16)
    bias_d = dp("bias", [O, 1], F32)
    bom_d = dp("bom", [27, 1], F32)
    dbias_d = dp("dbias", [36, 5], F32)
    mxlb_d = dp("mxlb", [NP, 1], F32)
    one125_d = dp("one125", [NP, 1], F32)
    out_d = dp("out", [O, P], BF16, out=True)

    omdram = nc.dram_tensor("omdram", [27, P], BF16).ap()
    tdram = nc.dram_tensor("tdram", [HH * N1], FP16).ap()
    adram = nc.dram_tensor("adram", [HH * ND * N1], BF16).ap()
    t_wv = tdram.rearrange("(r k c) -> k r c", k=K, c=WJ)       # write view
    t_rv = tdram.rearrange("(r c) -> r c", c=N1)                # read view
    a_wv = adram.rearrange("(r d k c) -> d k r c", d=ND, k=K, c=WJ)
    a_rv = adram.rearrange("(r d c) -> r d c", d=ND, c=N1)

    with tile.TileContext(nc) as tc:
        with (
            tc.tile_pool(name="small", bufs=1) as small,
            tc.tile_pool(name="big", bufs=1) as big,
            tc.tile_pool(name="plp", bufs=2) as plp,
            tc.tile_pool(name="io", bufs=3) as io,
            tc.tile_pool(name="rowp", bufs=3) as rowp,
            tc.tile_pool(name="vpool", bufs=5) as vpool,
            tc.tile_pool(name="ps", bufs=3, space="PSUM") as ps,
            tc.tile_pool(name="ps2", bufs=2, space="PSUM") as ps2,
        ):
            womr = small.tile([2 * C, 9 * 27], BF16)
            nc.sync.dma_start(out=womr[:], in_=womr_d[:])
            w2r = small.tile([C, K * O], BF16)
            nc.sync.dma_start(out=w2r[:], in_=w2r_d[:])
            jkt = small.tile([36, 2176], FP16)
            nc.sync.dma_start(out=jkt[:], in_=jkt_d[:])
            bias = small.tile([O, 1], F32)
            nc.sync.dma_start(out=bias[:], in_=bias_d[:])
            bom = small.tile([27, 1], F32)
            nc.sync.dma_start(out=bom[:], in_=bom_d[:])
            dbias = small.tile([36, 5], F32)
            nc.sync.dma_start(out=dbias[:], in_=dbias_d[:])
            mxlb = small.tile([NP, 1], F32)
            nc.sync.dma_start(out=mxlb[:], in_=mxlb_d[:])
            one125 = small.tile([NP, 1], F32)
            nc.sync.dma_start(out=one125[:], in_=one125_d[:])

            feat = big.tile([2 * C, RG * 130], BF16)
            nc.sync.dma_start(out=feat[:], in_=feat_d[:])
            xwb = big.tile([NP, XSL * NW * C], BF16)
            nc.sync.dma_start(out=xwb[:, 0:10 * NW * C],
                              in_=xwin_d[:, 0:10 * NW * C])
            loaded = 10

            # ---- 1. offset conv (bf16 moving, DVE bias-evac) ----
            for ch in range(16):
                pom = ps.tile([C, 1024], F32, tag="sup")
                for t9 in range(9):
                    dy, dx = t9 // 3, t9 % 3
                    base = feat[:, (ch * 4 + dy) * 130 + dx:
                                (ch * 4 + dy) * 130 + dx + 1]
                    rhs = _ap(base, [[130, 4], [1, 128]])
                    nc.tensor.matmul(pom[0:27, 0:512],
                                     womr[:, t9 * 27:(t9 + 1) * 27],
                                     rhs, start=(t9 == 0), stop=(t9 == 8))
                omc = io.tile([27, 512], BF16, tag="omc")
                nc.scalar.activation(omc[:], pom[0:27, 0:512], AF.Identity,
                                     bias=bom[:])
                nc.sync.dma_start(out=omdram[:, ch * 512:(ch + 1) * 512],
                                  in_=omc[:])
                if debug:
                    nc.sync.dma_start(
                        out=dbg["om"][:, ch * 512:(ch + 1) * 512], in_=omc[:])

            # ---- 2. hat planes [36, 2056/2176] ----
            dyf = big.tile([36, 2056], BF16)
            dxf = big.tile([36, 2056], BF16)
            msf = big.tile([36, 2056], BF16)
            for f in (dyf, dxf, msf):
                nc.vector.memset(f[:, 2048:2056], 0.0)
            for q in range(4):
                pix = slice(q * 2048, (q + 1) * 2048)
                nc.sync.dma_start(out=dyf[q * 9:(q + 1) * 9, 0:2048],
                                  in_=omdram[0:18:2, pix])
                nc.sync.dma_start(out=dxf[q * 9:(q + 1) * 9, 0:2048],
                                  in_=omdram[1:18:2, pix])
                nc.sync.dma_start(out=msf[q * 9:(q + 1) * 9, 0:2048],
                                  in_=omdram[18:27, pix])
            nc.scalar.activation(msf[:, 0:2048], msf[:, 0:2048], AF.Sigmoid,
                                 bias=dbias[:, 2:3])

            dxv = _ap(dxf[:], [[128, 16], [17, 8], [1, 17]])
            dyv = _ap(dyf[:], [[128, 16], [17, 8], [1, 17]])
            msv = _ap(msf[:], [[128, 16], [17, 8], [1, 17]])
            t136 = big.tile([36, 2176], FP16)
            nc.vector.tensor_tensor(
                t136[:].rearrange("p (a b c) -> p a b c", a=16, b=8, c=17),
                dxv, jkt[:].rearrange("p (a b c) -> p a b c",
                                      a=16, b=8, c=17), op=ALU.add)
            for q in range(4):
                nc.sync.dma_start(
                    out=t_wv[:, q * 16:(q + 1) * 16, :],
                    in_=t136[q * 9:(q + 1) * 9, :].rearrange(
                        "p (r c) -> p r c", r=16))
            if debug:
                nc.sync.dma_start(out=dbg["t136"][:], in_=t136[:])

            for d5 in range(ND):
                ab = plp.tile([36, 2176], BF16, tag="aplane")
                ab4 = ab[:].rearrange("p (a b c) -> p a b c", a=16, b=8, c=17)
                nc.scalar.activation(ab4, dyv, AF.Abs,
                                     bias=dbias[:, d5:d5 + 1], scale=1.0)
                nc.vector.tensor_scalar(ab[:], ab[:], 1.0, None,
                                        op0=ALU.subtract)
                abb = plp.tile([36, 2176], BF16, tag="aplaneb")
                abb4 = abb[:].rearrange("p (a b c) -> p a b c",
                                        a=16, b=8, c=17)
                nc.vector.tensor_tensor(abb4, ab4, msv, op=ALU.mult)
                nc.vector.tensor_scalar(abb[:], abb[:], -1.0, 0.0,
                                        op0=ALU.mult, op1=ALU.max)
                for q in range(4):
                    nc.sync.dma_start(
                        out=a_wv[d5, :, q * 16:(q + 1) * 16, :],
                        in_=abb[q * 9:(q + 1) * 9, :].rearrange(
                            "p (r c) -> p r c", r=16))
                if debug:
                    nc.sync.dma_start(
                        out=dbg["abb"][d5 * 36:(d5 + 1) * 36, :], in_=abb[:])

            # ---- 3-5. rows ----
            pend = []
            tri = {}

            def stage2(r):
                bl = r % 8
                idx = bl % 3 if bl < 6 else bl - 6
                if idx == 0:
                    tri["p"] = ps2.tile([O, 3 * WJ], F32, tag="pout",
                                        name="pout")
                    tri["r0"] = r
                pout = tri["p"]
                vt = vts.pop(r)
                for k9 in range(K):
                    mbase = vt[:, k9 * CW:k9 * CW + 1]
                    mv = _ap(mbase, [[K * CW, NW], [1, CW]])
                    nc.tensor.matmul(pout[:, idx * WJ:(idx + 1) * WJ],
                                     w2r[:, k9 * O:(k9 + 1) * O],
                                     mv, start=(k9 == 0), stop=(k9 == K - 1))
                if bl in (2, 5, 7):
                    n = idx + 1
                    osb = io.tile([O, 3 * 128], BF16, tag="osb")
                    sbase = pout[:, 0:1]
                    src = _ap(sbase, [[WJ, n], [1, 128]])
                    nc.scalar.activation(osb[:, 0:n * 128], src, AF.Identity,
                                         bias=bias[:])
                    nc.sync.dma_start(
                        out=out_d[:, tri["r0"] * 128:(r + 1) * 128],
                        in_=osb[:, 0:n * 128])

            vts = {}
            for r in range(HH):
                if r % 8 == 0 and r > 0:
                    need = min(r + 9, RG - 1)
                    while loaded <= need:
                        s0 = loaded % XSL
                        cnt = min(XSL - s0, need - loaded + 1)
                        nc.sync.dma_start(
                            out=xwb[:, s0 * 512:(s0 + cnt) * 512],
                            in_=xwin_d[:, loaded * 512:(loaded + cnt) * 512])
                        loaded += cnt
                if r % 4 == 0:
                    texp4 = rowp.tile([NP, 4 * N1], FP16, tag="texp")
                    nc.sync.dma_start(
                        out=texp4[:],
                        in_=tdram[r * N1:(r + 4) * N1]
                        .unsqueeze(0).broadcast_to([NP, 4 * N1]))
                    amt4 = rowp.tile([NP, 4 * N1], BF16, tag="amt")
                    for d5 in range(ND):
                        nc.sync.dma_start(
                            out=amt4[d5 * 25:(d5 + 1) * 25, :].rearrange(
                                "p (w c) -> p w c", w=4),
                            in_=bass.AP(adram.tensor,
                                        adram.offset + (r * ND + d5) * N1,
                                        [[0, 25], [ND * N1, 4], [1, N1]]))
                rs = (r % 4) * N1
                texp = texp4[:, rs:rs + N1]
                amt = amt4[:, rs:rs + N1]
                sab = rowp.tile([NP, N1], BF16, tag="sab")
                nc.scalar.activation(sab[:], texp, AF.Abs,
                                     bias=mxlb[:], scale=1.0)
                nc.vector.scalar_tensor_tensor(sab[:], sab[:], one125[:],
                                               amt, op0=ALU.subtract,
                                               op1=ALU.mult)
                mrow = rowp.tile([NP, N1], BF16, tag="mrow")
                nc.vector.tensor_scalar(mrow[:], sab[:], -1.0, 0.0,
                                        op0=ALU.mult, op1=ALU.max)
                if debug and r == 5:
                    nc.sync.dma_start(out=dbg["mrow"][:], in_=mrow[:])
                    nc.sync.dma_start(out=dbg["texp"][:], in_=texp)
                    nc.sync.dma_start(out=dbg["amt"][:], in_=amt)
                vt = vpool.tile([C, N1], BF16, tag="v")
                vts[r] = vt
                for h2 in range(2):
                    sup = ps.tile([C, 1024], F32, tag="sup")
                    for g in range(4):
                        w8 = h2 * 4 + g
                        off = (g // 2) * 512 + (g % 2) * 153
                        for ky in range(3):
                            slot = (r + ky) % XSL
                            st = xwb[:, slot * 512 + w8 * C:
                                     slot * 512 + (w8 + 1) * C]
                            mbase = mrow[:, (3 * ky) * WJ + w8 * CW:
                                         (3 * ky) * WJ + w8 * CW + 1]
                            mv = _ap(mbase, [[WJ, 3], [1, CW]])
                            nc.tensor.matmul(
                                sup[:, off + 51 * ky:off + 51 * (ky + 1)],
                                st, mv, start=True, stop=True)
                    sbase = sup[:, 0:1]
                    src = _ap(sbase, [[512, 2], [153, 2], [1, 153]])
                    dst = vt[:, h2 * 612:(h2 + 1) * 612]
                    if h2 == 0:
                        nc.scalar.activation(dst, src, AF.Copy, bias=0.0)
                    else:
                        nc.vector.tensor_copy(dst, src)
                if debug and r == 5:
                    nc.sync.dma_start(out=dbg["vt"][:], in_=vt[:])
                pend.append(r)
                if r >= 2:
                    stage2(pend.pop(0))
            while pend:
                stage2(pend.pop(0))
    nc.compile()
    return nc


def host_prep(input_feat, inter, weight, bias, w_om, b_om):
    import ml_dtypes
    bf = ml_dtypes.bfloat16
    maps = []
    womr = np.ascontiguousarray(
        w_om.transpose(1, 2, 3, 0).reshape(2 * C, 9 * 27)).astype(bf)
    w2r = np.ascontiguousarray(
        weight.reshape(O, C, K).transpose(1, 2, 0).reshape(C, K * O)
    ).astype(bf)
    kk = (np.arange(K) % 3).astype(np.float32)
    jj = np.arange(CW, dtype=np.float32)
    jkt = np.broadcast_to(
        3.0 + kk[:, None, None] + jj[None, None, :],
        (K, NW, CW)).reshape(1, N1)                      # [1, (k, w8, j)]
    # jkt tile layout: [36=(q,k), (r16, w8, j)] -> value dep on (k, w8, j)
    jkt_t = np.zeros((36, 2176), np.float32)
    kwj = (3.0 + kk[:, None, None] + 0 * np.arange(NW)[None, :, None]
           + jj[None, None, :])                          # [9, 8, 17]
    for q in range(4):
        for k in range(K):
            jkt_t[q * 9 + k] = np.tile(kwj[k].reshape(1, NW * CW),
                                       (16, 1)).reshape(-1)
    dbias = np.tile(-(np.arange(5, dtype=np.float32) - 2), (36, 1))
    mxlb = -(np.arange(NP) % 25).astype(np.float32)[:, None]
    for b in range(B):
        xpad = np.zeros((C, H + 6, 144), np.float32)
        xpad[:, 3:3 + H, 4:4 + W] = input_feat[b]
        featb = np.concatenate([input_feat[b], inter[b]], axis=0)
        for half in range(2):
            h0 = half * HH
            fs = np.zeros((2 * C, RG, 130), np.float32)
            r_lo, r_hi = max(0, h0 - 1), min(H, h0 + 65)
            fs[:, r_lo - (h0 - 1):r_hi - (h0 - 1), 1:129] = featb[:, r_lo:r_hi]
            xs = xpad[:, h0:h0 + 70, :]                    # [C, 70, 144]
            rows = (np.arange(RG)[:, None, None, None]
                    + np.arange(ND)[None, None, None, :])  # r + d
            cols = (17 * np.arange(NW)[None, :, None, None]
                    + np.arange(25)[None, None, :, None])
            xw = xs[:, rows, cols]                         # [C,66,NW,25,ND]
            xw = xw.transpose(4, 3, 1, 2, 0).reshape(NP, RG * NW * C)
            maps.append({
                "feat": fs.reshape(2 * C, RG * 130).astype(bf),
                "xwin": xw.astype(bf),
                "womr": womr, "w2r": w2r,
                "jkt": jkt_t.astype(np.float16),
                "bias": np.asarray(bias, np.float32).reshape(O, 1),
                "bom": np.asarray(b_om, np.float32).reshape(27, 1),
                "dbias": dbias,
                "mxlb": mxlb,
                "one125": np.ones((NP, 1), np.float32),
            })
    return maps


def kernel(input_feat, inter, weight, bias, w_om, b_om):
    if "nc" not in _cache:
        _cache["nc"] = build_bass()
    nc = _cache["nc"]
    maps = host_prep(np.asarray(input_feat, np.float32),
                     np.asarray(inter, np.float32),
                     np.asarray(weight, np.float32),
                     np.asarray(bias, np.float32),
                     np.asarray(w_om, np.float32),
                     np.asarray(b_om, np.float32))
    res = run_bass_kernel_spmd(nc, maps, list(range(NCORES)))
    out = np.zeros((B, O, H, W), np.float32)
    for ci in range(NCORES):
        b, half = ci // 2, ci % 2
        out[b, :, half * HH:(half + 1) * HH] = \
            res.results[ci]["out"].astype(np.float32).reshape(O, HH, W)
    return out


# revision 28
# speedup vs baseline: 2.2431x; 1.0132x over previous
"""DCNv2 (modulated deformable conv) Trainium2 kernel, v2.

8 cores = 4 batch samples x 2 image halves. Per core:
  1. Offset conv on PE (bf16 moving): om[27, 8192] -> omdram (bf16).
  2. Hat planes: A[d] = relu(1-|dy-(d-2)|)*sigmoid(m) on [36, 2176]
     tiles (ACT abs + DVE); t-plane = (dx + 3 + kx + j) in fp16,
     (k, w8, j) layout. Written to DRAM row-major so per-row
     broadcasts are single >=2KB-elem DMAs.
  3. Per row r: two broadcast DMAs (t-row -> 125 parts, A-row -> 25x
     replication), then M = relu((1-|t' - x_l|)*A):
     ACT Abs(texp, bias=-x_l) -> DVE stt (a-1)*A -> DVE relu-ts.
  4. Stage-1: 24 matmuls (125-contraction) into 2 psum supertiles
     [64, 1024] (bank-aligned 153-col regions); one strided Copy per
     supertile (ACT for half 0, DVE for half 1) into vt [64, 1224].
  5. Stage-2: per row 9 matmuls w2r[64,128].T @ vt-slices into
     row-triple psum [128, 3*136]; ACT adds bias on evac, bf16 out.
"""
import sys
sys.path.insert(0, "/opt/trn_rl_repo")
import numpy as np
import concourse.bass as bass
import concourse.tile as tile
from concourse import bacc, mybir
from concourse.bass_utils import run_bass_kernel_spmd

F32, BF16, FP16 = mybir.dt.float32, mybir.dt.bfloat16, mybir.dt.float16
ALU = mybir.AluOpType
AF = mybir.ActivationFunctionType

B, C, O, H, W = 4, 64, 128, 128, 128
K, HH = 9, 64
P = HH * W
CW, NW, ND, NP = 17, 8, 5, 125
WJ = NW * CW           # 136
N1 = K * WJ            # 1224
RG = HH + 2            # 66 xwin rowgroups
XSL = 16               # xwb ring slots
NCORES = 8

_cache = {}


def _ap(base, dims):
    """Manual AP: keep base partition dim, replace free dims."""
    return bass.AP(base.tensor, base.offset, [base.ap[0]] + dims)


def build_bass(debug=False):
    nc = bacc.Bacc("TRN2", target_bir_lowering=False, debug=False,
                   num_devices=NCORES)
    dp = lambda n, s, dt, out=False: nc.dram_tensor(
        n, s, dt, kind="ExternalOutput" if out else "ExternalInput").ap()
    dbg = {}
    if debug:
        dbg["om"] = dp("dbg_om", [27, P], BF16, out=True)
        dbg["t136"] = dp("dbg_t136", [36, 2176], FP16, out=True)
        dbg["abb"] = dp("dbg_abb", [ND * 36, 2176], BF16, out=True)
        dbg["mrow"] = dp("dbg_mrow", [NP, N1], BF16, out=True)
        dbg["vt"] = dp("dbg_vt", [C, N1], BF16, out=True)
        dbg["texp"] = dp("dbg_texp", [NP, N1], FP16, out=True)
        dbg["amt"] = dp("dbg_amt", [NP, N1], BF16, out=True)

    feat_d = dp("feat", [2 * C, RG * 130], BF16)
    xwin_d = dp("xwin", [NP, RG * NW * C], BF16)
    womr_d = dp("womr", [2 * C, 9 * 27], BF16)
    w2r_d = dp("w2r", [C, K * O], BF16)
    jkt_d = dp("jkt", [36, 2176], FP16)
    bias_d = dp("bias", [O, 1], F32)
    bom_d = dp("bom", [27, 1], F32)
    dbias_d = dp("dbias", [36, 5], F32)
    mxlb_d = dp("mxlb", [NP, 1], F32)
    one125_d = dp("one125", [NP, 1], F32)
    out_d = dp("out", [O, P], BF16, out=True)

    omdram = nc.dram_tensor("omdram", [27, P], BF16).ap()
    tdram = nc.dram_tensor("tdram", [HH * N1], FP16).ap()
    adram = nc.dram_tensor("adram", [HH * ND * N1], BF16).ap()
    t_wv = tdram.rearrange("(r k c) -> k r c", k=K, c=WJ)       # write view
    t_rv = tdram.rearrange("(r c) -> r c", c=N1)                # read view
    a_wv = adram.rearrange("(r d k c) -> d k r c", d=ND, k=K, c=WJ)
    a_rv = adram.rearrange("(r d c) -> r d c", d=ND, c=N1)

    with tile.TileContext(nc) as tc:
        with (
            tc.tile_pool(name="small", bufs=1) as small,
            tc.tile_pool(name="big", bufs=1) as big,
            tc.tile_pool(name="plp", bufs=2) as plp,
            tc.tile_pool(name="io", bufs=3) as io,
            tc.tile_pool(name="rowp", bufs=3) as rowp,
            tc.tile_pool(name="mrp", bufs=6) as mrp,
            tc.tile_pool(name="vpool", bufs=5) as vpool,
            tc.tile_pool(name="ps", bufs=3, space="PSUM") as ps,
            tc.tile_pool(name="ps2", bufs=2, space="PSUM") as ps2,
        ):
            womr = small.tile([2 * C, 9 * 27], BF16)
            nc.sync.dma_start(out=womr[:], in_=womr_d[:])
            w2r = small.tile([C, K * O], BF16)
            nc.sync.dma_start(out=w2r[:], in_=w2r_d[:])
            jkt = small.tile([36, 2176], FP16)
            nc.sync.dma_start(out=jkt[:], in_=jkt_d[:])
            bias = small.tile([O, 1], F32)
            nc.sync.dma_start(out=bias[:], in_=bias_d[:])
            bom = small.tile([27, 1], F32)
            nc.sync.dma_start(out=bom[:], in_=bom_d[:])
            dbias = small.tile([36, 5], F32)
            nc.sync.dma_start(out=dbias[:], in_=dbias_d[:])
            mxlb = small.tile([NP, 1], F32)
            nc.sync.dma_start(out=mxlb[:], in_=mxlb_d[:])
            one125 = small.tile([NP, 1], F32)
            nc.sync.dma_start(out=one125[:], in_=one125_d[:])

            feat = big.tile([2 * C, RG * 130], BF16)
            nc.sync.dma_start(out=feat[:], in_=feat_d[:])
            xwb = big.tile([NP, XSL * NW * C], BF16)
            nc.sync.dma_start(out=xwb[:, 0:10 * NW * C],
                              in_=xwin_d[:, 0:10 * NW * C])
            loaded = 10

            # ---- 1. offset conv (bf16 moving, DVE bias-evac) ----
            for ch in range(16):
                pom = ps.tile([C, 1024], F32, tag="sup")
                for t9 in range(9):
                    dy, dx = t9 // 3, t9 % 3
                    base = feat[:, (ch * 4 + dy) * 130 + dx:
                                (ch * 4 + dy) * 130 + dx + 1]
                    rhs = _ap(base, [[130, 4], [1, 128]])
                    nc.tensor.matmul(pom[0:27, 0:512],
                                     womr[:, t9 * 27:(t9 + 1) * 27],
                                     rhs, start=(t9 == 0), stop=(t9 == 8))
                omc = io.tile([27, 512], BF16, tag="omc")
                nc.scalar.activation(omc[:], pom[0:27, 0:512], AF.Identity,
                                     bias=bom[:])
                nc.sync.dma_start(out=omdram[:, ch * 512:(ch + 1) * 512],
                                  in_=omc[:])
                if debug:
                    nc.sync.dma_start(
                        out=dbg["om"][:, ch * 512:(ch + 1) * 512], in_=omc[:])

            # ---- 2. hat planes [36, 2056/2176] ----
            dyf = big.tile([36, 2056], BF16)
            dxf = big.tile([36, 2056], BF16)
            msf = big.tile([36, 2056], BF16)
            for f in (dyf, dxf, msf):
                nc.vector.memset(f[:, 2048:2056], 0.0)
            for q in range(4):
                pix = slice(q * 2048, (q + 1) * 2048)
                nc.sync.dma_start(out=dyf[q * 9:(q + 1) * 9, 0:2048],
                                  in_=omdram[0:18:2, pix])
                nc.sync.dma_start(out=dxf[q * 9:(q + 1) * 9, 0:2048],
                                  in_=omdram[1:18:2, pix])
                nc.sync.dma_start(out=msf[q * 9:(q + 1) * 9, 0:2048],
                                  in_=omdram[18:27, pix])
            nc.scalar.activation(msf[:, 0:2048], msf[:, 0:2048], AF.Sigmoid,
                                 bias=dbias[:, 2:3])

            dxv = _ap(dxf[:], [[128, 16], [17, 8], [1, 17]])
            dyv = _ap(dyf[:], [[128, 16], [17, 8], [1, 17]])
            msv = _ap(msf[:], [[128, 16], [17, 8], [1, 17]])
            t136 = big.tile([36, 2176], FP16)
            nc.vector.tensor_tensor(
                t136[:].rearrange("p (a b c) -> p a b c", a=16, b=8, c=17),
                dxv, jkt[:].rearrange("p (a b c) -> p a b c",
                                      a=16, b=8, c=17), op=ALU.add)
            for q in range(4):
                nc.sync.dma_start(
                    out=t_wv[:, q * 16:(q + 1) * 16, :],
                    in_=t136[q * 9:(q + 1) * 9, :].rearrange(
                        "p (r c) -> p r c", r=16))
            if debug:
                nc.sync.dma_start(out=dbg["t136"][:], in_=t136[:])

            for d5 in range(ND):
                ab = plp.tile([36, 2176], BF16, tag="aplane")
                ab4 = ab[:].rearrange("p (a b c) -> p a b c", a=16, b=8, c=17)
                nc.scalar.activation(ab4, dyv, AF.Abs,
                                     bias=dbias[:, d5:d5 + 1], scale=1.0)
                nc.vector.tensor_scalar(ab[:], ab[:], 1.0, None,
                                        op0=ALU.subtract)
                abb = plp.tile([36, 2176], BF16, tag="aplaneb")
                abb4 = abb[:].rearrange("p (a b c) -> p a b c",
                                        a=16, b=8, c=17)
                nc.vector.tensor_tensor(abb4, ab4, msv, op=ALU.mult)
                nc.vector.tensor_scalar(abb[:], abb[:], -1.0, 0.0,
                                        op0=ALU.mult, op1=ALU.max)
                for q in range(4):
                    nc.sync.dma_start(
                        out=a_wv[d5, :, q * 16:(q + 1) * 16, :],
                        in_=abb[q * 9:(q + 1) * 9, :].rearrange(
                            "p (r c) -> p r c", r=16))
                if debug:
                    nc.sync.dma_start(
                        out=dbg["abb"][d5 * 36:(d5 + 1) * 36, :], in_=abb[:])

            # ---- 3-5. rows ----
            pend = []
            tri = {}

            def stage2(r):
                bl = r % 8
                idx = bl % 3 if bl < 6 else bl - 6
                if idx == 0:
                    tri["p"] = ps2.tile([O, 3 * WJ], F32, tag="pout",
                                        name="pout")
                    tri["r0"] = r
                pout = tri["p"]
                vt = vts.pop(r)
                for k9 in range(K):
                    mbase = vt[:, k9 * CW:k9 * CW + 1]
                    mv = _ap(mbase, [[K * CW, NW], [1, CW]])
                    nc.tensor.matmul(pout[:, idx * WJ:(idx + 1) * WJ],
                                     w2r[:, k9 * O:(k9 + 1) * O],
                                     mv, start=(k9 == 0), stop=(k9 == K - 1))
                if bl in (2, 5, 7):
                    n = idx + 1
                    osb = io.tile([O, 3 * 128], BF16, tag="osb")
                    sbase = pout[:, 0:1]
                    src = _ap(sbase, [[WJ, n], [1, 128]])
                    nc.scalar.activation(osb[:, 0:n * 128], src, AF.Identity,
                                         bias=bias[:])
                    nc.sync.dma_start(
                        out=out_d[:, tri["r0"] * 128:(r + 1) * 128],
                        in_=osb[:, 0:n * 128])

            vts = {}
            for r in range(HH):
                if r % 8 == 0 and r > 0:
                    need = min(r + 9, RG - 1)
                    while loaded <= need:
                        s0 = loaded % XSL
                        cnt = min(XSL - s0, need - loaded + 1)
                        nc.sync.dma_start(
                            out=xwb[:, s0 * 512:(s0 + cnt) * 512],
                            in_=xwin_d[:, loaded * 512:(loaded + cnt) * 512])
                        loaded += cnt
                if r % 4 == 0:
                    texp4 = rowp.tile([NP, 4 * N1], FP16, tag="texp")
                    nc.sync.dma_start(
                        out=texp4[:],
                        in_=tdram[r * N1:(r + 4) * N1]
                        .unsqueeze(0).broadcast_to([NP, 4 * N1]))
                    amt4 = rowp.tile([NP, 4 * N1], BF16, tag="amt")
                    for d5 in range(ND):
                        nc.sync.dma_start(
                            out=amt4[d5 * 25:(d5 + 1) * 25, :].rearrange(
                                "p (w c) -> p w c", w=4),
                            in_=bass.AP(adram.tensor,
                                        adram.offset + (r * ND + d5) * N1,
                                        [[0, 25], [ND * N1, 4], [1, N1]]))
                rs = (r % 4) * N1
                texp = texp4[:, rs:rs + N1]
                amt = amt4[:, rs:rs + N1]
                sab = mrp.tile([NP, N1], BF16, tag="sab")
                nc.scalar.activation(sab[:], texp, AF.Abs,
                                     bias=mxlb[:], scale=1.0)
                nc.vector.scalar_tensor_tensor(sab[:], sab[:], one125[:],
                                               amt, op0=ALU.subtract,
                                               op1=ALU.mult)
                mrow = mrp.tile([NP, N1], BF16, tag="mrow")
                nc.vector.tensor_scalar(mrow[:], sab[:], -1.0, 0.0,
                                        op0=ALU.mult, op1=ALU.max)
                if debug and r == 5:
                    nc.sync.dma_start(out=dbg["mrow"][:], in_=mrow[:])
                    nc.sync.dma_start(out=dbg["texp"][:], in_=texp)
                    nc.sync.dma_start(out=dbg["amt"][:], in_=amt)
                vt = vpool.tile([C, N1], BF16, tag="v")
                vts[r] = vt
                for h2 in range(2):
                    sup = ps.tile([C, 1024], F32, tag="sup")
                    for g in range(4):
                        w8 = h2 * 4 + g
                        off = (g // 2) * 512 + (g % 2) * 153
                        for ky in range(3):
                            slot = (r + ky) % XSL
                            st = xwb[:, slot * 512 + w8 * C:
                                     slot * 512 + (w8 + 1) * C]
                            mbase = mrow[:, (3 * ky) * WJ + w8 * CW:
                                         (3 * ky) * WJ + w8 * CW + 1]
                            mv = _ap(mbase, [[WJ, 3], [1, CW]])
                            nc.tensor.matmul(
                                sup[:, off + 51 * ky:off + 51 * (ky + 1)],
                                st, mv, start=True, stop=True)
                    sbase = sup[:, 0:1]
                    src = _ap(sbase, [[512, 2], [153, 2], [1, 153]])
                    dst = vt[:, h2 * 612:(h2 + 1) * 612]
                    if h2 == 0:
                        nc.scalar.activation(dst, src, AF.Copy, bias=0.0)
                    else:
                        nc.vector.tensor_copy(dst, src)
                if debug and r == 5:
                    nc.sync.dma_start(out=dbg["vt"][:], in_=vt[:])
                pend.append(r)
                if r >= 2:
                    stage2(pend.pop(0))
            while pend:
                stage2(pend.pop(0))
    nc.compile()
    return nc


def host_prep(input_feat, inter, weight, bias, w_om, b_om):
    import ml_dtypes
    bf = ml_dtypes.bfloat16
    maps = []
    womr = np.ascontiguousarray(
        w_om.transpose(1, 2, 3, 0).reshape(2 * C, 9 * 27)).astype(bf)
    w2r = np.ascontiguousarray(
        weight.reshape(O, C, K).transpose(1, 2, 0).reshape(C, K * O)
    ).astype(bf)
    kk = (np.arange(K) % 3).astype(np.float32)
    jj = np.arange(CW, dtype=np.float32)
    jkt = np.broadcast_to(
        3.0 + kk[:, None, None] + jj[None, None, :],
        (K, NW, CW)).reshape(1, N1)                      # [1, (k, w8, j)]
    # jkt tile layout: [36=(q,k), (r16, w8, j)] -> value dep on (k, w8, j)
    jkt_t = np.zeros((36, 2176), np.float32)
    kwj = (3.0 + kk[:, None, None] + 0 * np.arange(NW)[None, :, None]
           + jj[None, None, :])                          # [9, 8, 17]
    for q in range(4):
        for k in range(K):
            jkt_t[q * 9 + k] = np.tile(kwj[k].reshape(1, NW * CW),
                                       (16, 1)).reshape(-1)
    dbias = np.tile(-(np.arange(5, dtype=np.float32) - 2), (36, 1))
    mxlb = -(np.arange(NP) % 25).astype(np.float32)[:, None]
    for b in range(B):
        xpad = np.zeros((C, H + 6, 144), np.float32)
        xpad[:, 3:3 + H, 4:4 + W] = input_feat[b]
        featb = np.concatenate([input_feat[b], inter[b]], axis=0)
        for half in range(2):
            h0 = half * HH
            fs = np.zeros((2 * C, RG, 130), np.float32)
            r_lo, r_hi = max(0, h0 - 1), min(H, h0 + 65)
            fs[:, r_lo - (h0 - 1):r_hi - (h0 - 1), 1:129] = featb[:, r_lo:r_hi]
            xs = xpad[:, h0:h0 + 70, :]                    # [C, 70, 144]
            rows = (np.arange(RG)[:, None, None, None]
                    + np.arange(ND)[None, None, None, :])  # r + d
            cols = (17 * np.arange(NW)[None, :, None, None]
                    + np.arange(25)[None, None, :, None])
            xw = xs[:, rows, cols]                         # [C,66,NW,25,ND]
            xw = xw.transpose(4, 3, 1, 2, 0).reshape(NP, RG * NW * C)
            maps.append({
                "feat": fs.reshape(2 * C, RG * 130).astype(bf),
                "xwin": xw.astype(bf),
                "womr": womr, "w2r": w2r,
                "jkt": jkt_t.astype(np.float16),
                "bias": np.asarray(bias, np.float32).reshape(O, 1),
                "bom": np.asarray(b_om, np.float32).reshape(27, 1),
                "dbias": dbias,
                "mxlb": mxlb,
                "one125": np.ones((NP, 1), np.float32),
            })
    return maps


def kernel(input_feat, inter, weight, bias, w_om, b_om):
    if "nc" not in _cache:
        _cache["nc"] = build_bass()
    nc = _cache["nc"]
    maps = host_prep(np.asarray(input_feat, np.float32),
                     np.asarray(inter, np.float32),
                     np.asarray(weight, np.float32),
                     np.asarray(bias, np.float32),
                     np.asarray(w_om, np.float32),
                     np.asarray(b_om, np.float32))
    res = run_bass_kernel_spmd(nc, maps, list(range(NCORES)))
    out = np.zeros((B, O, H, W), np.float32)
    for ci in range(NCORES):
        b, half = ci // 2, ci % 2
        out[b, :, half * HH:(half + 1) * HH] = \
            res.results[ci]["out"].astype(np.float32).reshape(O, HH, W)
    return out


# revision 29
# speedup vs baseline: 2.2464x; 1.0015x over previous
"""DCNv2 (modulated deformable conv) Trainium2 kernel, v2.

8 cores = 4 batch samples x 2 image halves. Per core:
  1. Offset conv on PE (bf16 moving): om[27, 8192] -> omdram (bf16).
  2. Hat planes: A[d] = relu(1-|dy-(d-2)|)*sigmoid(m) on [36, 2176]
     tiles (ACT abs + DVE); t-plane = (dx + 3 + kx + j) in fp16,
     (k, w8, j) layout. Written to DRAM row-major so per-row
     broadcasts are single >=2KB-elem DMAs.
  3. Per row r: two broadcast DMAs (t-row -> 125 parts, A-row -> 25x
     replication), then M = relu((1-|t' - x_l|)*A):
     ACT Abs(texp, bias=-x_l) -> DVE stt (a-1)*A -> DVE relu-ts.
  4. Stage-1: 24 matmuls (125-contraction) into 2 psum supertiles
     [64, 1024] (bank-aligned 153-col regions); one strided Copy per
     supertile (ACT for half 0, DVE for half 1) into vt [64, 1224].
  5. Stage-2: per row 9 matmuls w2r[64,128].T @ vt-slices into
     row-triple psum [128, 3*136]; ACT adds bias on evac, bf16 out.
"""
import sys
sys.path.insert(0, "/opt/trn_rl_repo")
import numpy as np
import concourse.bass as bass
import concourse.tile as tile
from concourse import bacc, mybir
from concourse.bass_utils import run_bass_kernel_spmd

F32, BF16, FP16 = mybir.dt.float32, mybir.dt.bfloat16, mybir.dt.float16
ALU = mybir.AluOpType
AF = mybir.ActivationFunctionType

B, C, O, H, W = 4, 64, 128, 128, 128
K, HH = 9, 64
P = HH * W
CW, NW, ND, NP = 17, 8, 5, 125
WJ = NW * CW           # 136
N1 = K * WJ            # 1224
RG = HH + 2            # 66 xwin rowgroups
XSL = 16               # xwb ring slots
NCORES = 8

_cache = {}


def _ap(base, dims):
    """Manual AP: keep base partition dim, replace free dims."""
    return bass.AP(base.tensor, base.offset, [base.ap[0]] + dims)


def build_bass(debug=False):
    nc = bacc.Bacc("TRN2", target_bir_lowering=False, debug=False,
                   num_devices=NCORES)
    dp = lambda n, s, dt, out=False: nc.dram_tensor(
        n, s, dt, kind="ExternalOutput" if out else "ExternalInput").ap()
    dbg = {}
    if debug:
        dbg["om"] = dp("dbg_om", [27, P], BF16, out=True)
        dbg["t136"] = dp("dbg_t136", [36, 2176], FP16, out=True)
        dbg["abb"] = dp("dbg_abb", [ND * 36, 2176], BF16, out=True)
        dbg["mrow"] = dp("dbg_mrow", [NP, N1], BF16, out=True)
        dbg["vt"] = dp("dbg_vt", [C, N1], BF16, out=True)
        dbg["texp"] = dp("dbg_texp", [NP, N1], FP16, out=True)
        dbg["amt"] = dp("dbg_amt", [NP, N1], BF16, out=True)

    feat_d = dp("feat", [2 * C, RG * 130], BF16)
    xwin_d = dp("xwin", [NP, RG * NW * C], BF16)
    womr_d = dp("womr", [2 * C, 9 * 27], BF16)
    w2r_d = dp("w2r", [C, K * O], BF16)
    jkt_d = dp("jkt", [36, 2176], FP16)
    bias_d = dp("bias", [O, 1], F32)
    bom_d = dp("bom", [27, 1], F32)
    dbias_d = dp("dbias", [36, 5], F32)
    mxlb_d = dp("mxlb", [NP, 1], F32)
    one125_d = dp("one125", [NP, 1], F32)
    out_d = dp("out", [O, P], BF16, out=True)

    omdram = nc.dram_tensor("omdram", [27, P], BF16).ap()
    tdram = nc.dram_tensor("tdram", [HH * N1], FP16).ap()
    adram = nc.dram_tensor("adram", [HH * ND * N1], BF16).ap()
    t_wv = tdram.rearrange("(r k c) -> k r c", k=K, c=WJ)       # write view
    t_rv = tdram.rearrange("(r c) -> r c", c=N1)                # read view
    a_wv = adram.rearrange("(r d k c) -> d k r c", d=ND, k=K, c=WJ)
    a_rv = adram.rearrange("(r d c) -> r d c", d=ND, c=N1)

    with tile.TileContext(nc) as tc:
        with (
            tc.tile_pool(name="small", bufs=1) as small,
            tc.tile_pool(name="big", bufs=1) as big,
            tc.tile_pool(name="plp", bufs=2) as plp,
            tc.tile_pool(name="io", bufs=4) as io,
            tc.tile_pool(name="rowp", bufs=3) as rowp,
            tc.tile_pool(name="mrp", bufs=8) as mrp,
            tc.tile_pool(name="vpool", bufs=6) as vpool,
            tc.tile_pool(name="ps", bufs=3, space="PSUM") as ps,
            tc.tile_pool(name="ps2", bufs=2, space="PSUM") as ps2,
        ):
            womr = small.tile([2 * C, 9 * 27], BF16)
            nc.sync.dma_start(out=womr[:], in_=womr_d[:])
            w2r = small.tile([C, K * O], BF16)
            nc.sync.dma_start(out=w2r[:], in_=w2r_d[:])
            jkt = small.tile([36, 2176], FP16)
            nc.sync.dma_start(out=jkt[:], in_=jkt_d[:])
            bias = small.tile([O, 1], F32)
            nc.sync.dma_start(out=bias[:], in_=bias_d[:])
            bom = small.tile([27, 1], F32)
            nc.sync.dma_start(out=bom[:], in_=bom_d[:])
            dbias = small.tile([36, 5], F32)
            nc.sync.dma_start(out=dbias[:], in_=dbias_d[:])
            mxlb = small.tile([NP, 1], F32)
            nc.sync.dma_start(out=mxlb[:], in_=mxlb_d[:])
            one125 = small.tile([NP, 1], F32)
            nc.sync.dma_start(out=one125[:], in_=one125_d[:])

            feat = big.tile([2 * C, RG * 130], BF16)
            nc.sync.dma_start(out=feat[:], in_=feat_d[:])
            xwb = big.tile([NP, XSL * NW * C], BF16)
            nc.sync.dma_start(out=xwb[:, 0:10 * NW * C],
                              in_=xwin_d[:, 0:10 * NW * C])
            loaded = 10

            # ---- 1. offset conv (bf16 moving, DVE bias-evac) ----
            for ch in range(16):
                pom = ps.tile([C, 1024], F32, tag="sup")
                for t9 in range(9):
                    dy, dx = t9 // 3, t9 % 3
                    base = feat[:, (ch * 4 + dy) * 130 + dx:
                                (ch * 4 + dy) * 130 + dx + 1]
                    rhs = _ap(base, [[130, 4], [1, 128]])
                    nc.tensor.matmul(pom[0:27, 0:512],
                                     womr[:, t9 * 27:(t9 + 1) * 27],
                                     rhs, start=(t9 == 0), stop=(t9 == 8))
                omc = io.tile([27, 512], BF16, tag="omc")
                nc.scalar.activation(omc[:], pom[0:27, 0:512], AF.Identity,
                                     bias=bom[:])
                nc.sync.dma_start(out=omdram[:, ch * 512:(ch + 1) * 512],
                                  in_=omc[:])
                if debug:
                    nc.sync.dma_start(
                        out=dbg["om"][:, ch * 512:(ch + 1) * 512], in_=omc[:])

            # ---- 2. hat planes [36, 2056/2176] ----
            dyf = big.tile([36, 2056], BF16)
            dxf = big.tile([36, 2056], BF16)
            msf = big.tile([36, 2056], BF16)
            for f in (dyf, dxf, msf):
                nc.vector.memset(f[:, 2048:2056], 0.0)
            for q in range(4):
                pix = slice(q * 2048, (q + 1) * 2048)
                nc.sync.dma_start(out=dyf[q * 9:(q + 1) * 9, 0:2048],
                                  in_=omdram[0:18:2, pix])
                nc.sync.dma_start(out=dxf[q * 9:(q + 1) * 9, 0:2048],
                                  in_=omdram[1:18:2, pix])
                nc.sync.dma_start(out=msf[q * 9:(q + 1) * 9, 0:2048],
                                  in_=omdram[18:27, pix])
            nc.scalar.activation(msf[:, 0:2048], msf[:, 0:2048], AF.Sigmoid,
                                 bias=dbias[:, 2:3])

            dxv = _ap(dxf[:], [[128, 16], [17, 8], [1, 17]])
            dyv = _ap(dyf[:], [[128, 16], [17, 8], [1, 17]])
            msv = _ap(msf[:], [[128, 16], [17, 8], [1, 17]])
            t136 = big.tile([36, 2176], FP16)
            nc.vector.tensor_tensor(
                t136[:].rearrange("p (a b c) -> p a b c", a=16, b=8, c=17),
                dxv, jkt[:].rearrange("p (a b c) -> p a b c",
                                      a=16, b=8, c=17), op=ALU.add)
            for q in range(4):
                nc.sync.dma_start(
                    out=t_wv[:, q * 16:(q + 1) * 16, :],
                    in_=t136[q * 9:(q + 1) * 9, :].rearrange(
                        "p (r c) -> p r c", r=16))
            if debug:
                nc.sync.dma_start(out=dbg["t136"][:], in_=t136[:])

            for d5 in range(ND):
                ab = plp.tile([36, 2176], BF16, tag="aplane")
                ab4 = ab[:].rearrange("p (a b c) -> p a b c", a=16, b=8, c=17)
                nc.scalar.activation(ab4, dyv, AF.Abs,
                                     bias=dbias[:, d5:d5 + 1], scale=1.0)
                nc.vector.tensor_scalar(ab[:], ab[:], 1.0, None,
                                        op0=ALU.subtract)
                abb = plp.tile([36, 2176], BF16, tag="aplaneb")
                abb4 = abb[:].rearrange("p (a b c) -> p a b c",
                                        a=16, b=8, c=17)
                nc.vector.tensor_tensor(abb4, ab4, msv, op=ALU.mult)
                nc.vector.tensor_scalar(abb[:], abb[:], -1.0, 0.0,
                                        op0=ALU.mult, op1=ALU.max)
                for q in range(4):
                    nc.sync.dma_start(
                        out=a_wv[d5, :, q * 16:(q + 1) * 16, :],
                        in_=abb[q * 9:(q + 1) * 9, :].rearrange(
                            "p (r c) -> p r c", r=16))
                if debug:
                    nc.sync.dma_start(
                        out=dbg["abb"][d5 * 36:(d5 + 1) * 36, :], in_=abb[:])

            # ---- 3-5. rows ----
            pend = []
            tri = {}

            def stage2(r):
                bl = r % 8
                idx = bl % 3 if bl < 6 else bl - 6
                if idx == 0:
                    tri["p"] = ps2.tile([O, 3 * WJ], F32, tag="pout",
                                        name="pout")
                    tri["r0"] = r
                pout = tri["p"]
                vt = vts.pop(r)
                for k9 in range(K):
                    mbase = vt[:, k9 * CW:k9 * CW + 1]
                    mv = _ap(mbase, [[K * CW, NW], [1, CW]])
                    nc.tensor.matmul(pout[:, idx * WJ:(idx + 1) * WJ],
                                     w2r[:, k9 * O:(k9 + 1) * O],
                                     mv, start=(k9 == 0), stop=(k9 == K - 1))
                if bl in (2, 5, 7):
                    n = idx + 1
                    osb = io.tile([O, 3 * 128], BF16, tag="osb")
                    sbase = pout[:, 0:1]
                    src = _ap(sbase, [[WJ, n], [1, 128]])
                    nc.scalar.activation(osb[:, 0:n * 128], src, AF.Identity,
                                         bias=bias[:])
                    nc.sync.dma_start(
                        out=out_d[:, tri["r0"] * 128:(r + 1) * 128],
                        in_=osb[:, 0:n * 128])

            vts = {}
            for r in range(HH):
                if r % 8 == 0 and r > 0:
                    need = min(r + 9, RG - 1)
                    while loaded <= need:
                        s0 = loaded % XSL
                        cnt = min(XSL - s0, need - loaded + 1)
                        nc.sync.dma_start(
                            out=xwb[:, s0 * 512:(s0 + cnt) * 512],
                            in_=xwin_d[:, loaded * 512:(loaded + cnt) * 512])
                        loaded += cnt
                if r % 4 == 0:
                    texp4 = rowp.tile([NP, 4 * N1], FP16, tag="texp")
                    nc.sync.dma_start(
                        out=texp4[:],
                        in_=tdram[r * N1:(r + 4) * N1]
                        .unsqueeze(0).broadcast_to([NP, 4 * N1]))
                    amt4 = rowp.tile([NP, 4 * N1], BF16, tag="amt")
                    for d5 in range(ND):
                        nc.sync.dma_start(
                            out=amt4[d5 * 25:(d5 + 1) * 25, :].rearrange(
                                "p (w c) -> p w c", w=4),
                            in_=bass.AP(adram.tensor,
                                        adram.offset + (r * ND + d5) * N1,
                                        [[0, 25], [ND * N1, 4], [1, N1]]))
                rs = (r % 4) * N1
                texp = texp4[:, rs:rs + N1]
                amt = amt4[:, rs:rs + N1]
                sab = mrp.tile([NP, N1], BF16, tag="sab")
                nc.scalar.activation(sab[:], texp, AF.Abs,
                                     bias=mxlb[:], scale=1.0)
                nc.vector.scalar_tensor_tensor(sab[:], sab[:], one125[:],
                                               amt, op0=ALU.subtract,
                                               op1=ALU.mult)
                mrow = mrp.tile([NP, N1], BF16, tag="mrow")
                nc.vector.tensor_scalar(mrow[:], sab[:], -1.0, 0.0,
                                        op0=ALU.mult, op1=ALU.max)
                if debug and r == 5:
                    nc.sync.dma_start(out=dbg["mrow"][:], in_=mrow[:])
                    nc.sync.dma_start(out=dbg["texp"][:], in_=texp)
                    nc.sync.dma_start(out=dbg["amt"][:], in_=amt)
                vt = vpool.tile([C, N1], BF16, tag="v")
                vts[r] = vt
                for h2 in range(2):
                    sup = ps.tile([C, 1024], F32, tag="sup")
                    for g in range(4):
                        w8 = h2 * 4 + g
                        off = (g // 2) * 512 + (g % 2) * 153
                        for ky in range(3):
                            slot = (r + ky) % XSL
                            st = xwb[:, slot * 512 + w8 * C:
                                     slot * 512 + (w8 + 1) * C]
                            mbase = mrow[:, (3 * ky) * WJ + w8 * CW:
                                         (3 * ky) * WJ + w8 * CW + 1]
                            mv = _ap(mbase, [[WJ, 3], [1, CW]])
                            nc.tensor.matmul(
                                sup[:, off + 51 * ky:off + 51 * (ky + 1)],
                                st, mv, start=True, stop=True)
                    sbase = sup[:, 0:1]
                    src = _ap(sbase, [[512, 2], [153, 2], [1, 153]])
                    dst = vt[:, h2 * 612:(h2 + 1) * 612]
                    if h2 == 0:
                        nc.scalar.activation(dst, src, AF.Copy, bias=0.0)
                    else:
                        nc.vector.tensor_copy(dst, src)
                if debug and r == 5:
                    nc.sync.dma_start(out=dbg["vt"][:], in_=vt[:])
                pend.append(r)
                if r >= 2:
                    stage2(pend.pop(0))
            while pend:
                stage2(pend.pop(0))
    nc.compile()
    return nc


def host_prep(input_feat, inter, weight, bias, w_om, b_om):
    import ml_dtypes
    bf = ml_dtypes.bfloat16
    maps = []
    womr = np.ascontiguousarray(
        w_om.transpose(1, 2, 3, 0).reshape(2 * C, 9 * 27)).astype(bf)
    w2r = np.ascontiguousarray(
        weight.reshape(O, C, K).transpose(1, 2, 0).reshape(C, K * O)
    ).astype(bf)
    kk = (np.arange(K) % 3).astype(np.float32)
    jj = np.arange(CW, dtype=np.float32)
    jkt = np.broadcast_to(
        3.0 + kk[:, None, None] + jj[None, None, :],
        (K, NW, CW)).reshape(1, N1)                      # [1, (k, w8, j)]
    # jkt tile layout: [36=(q,k), (r16, w8, j)] -> value dep on (k, w8, j)
    jkt_t = np.zeros((36, 2176), np.float32)
    kwj = (3.0 + kk[:, None, None] + 0 * np.arange(NW)[None, :, None]
           + jj[None, None, :])                          # [9, 8, 17]
    for q in range(4):
        for k in range(K):
            jkt_t[q * 9 + k] = np.tile(kwj[k].reshape(1, NW * CW),
                                       (16, 1)).reshape(-1)
    dbias = np.tile(-(np.arange(5, dtype=np.float32) - 2), (36, 1))
    mxlb = -(np.arange(NP) % 25).astype(np.float32)[:, None]
    for b in range(B):
        xpad = np.zeros((C, H + 6, 144), np.float32)
        xpad[:, 3:3 + H, 4:4 + W] = input_feat[b]
        featb = np.concatenate([input_feat[b], inter[b]], axis=0)
        for half in range(2):
            h0 = half * HH
            fs = np.zeros((2 * C, RG, 130), np.float32)
            r_lo, r_hi = max(0, h0 - 1), min(H, h0 + 65)
            fs[:, r_lo - (h0 - 1):r_hi - (h0 - 1), 1:129] = featb[:, r_lo:r_hi]
            xs = xpad[:, h0:h0 + 70, :]                    # [C, 70, 144]
            rows = (np.arange(RG)[:, None, None, None]
                    + np.arange(ND)[None, None, None, :])  # r + d
            cols = (17 * np.arange(NW)[None, :, None, None]
                    + np.arange(25)[None, None, :, None])
            xw = xs[:, rows, cols]                         # [C,66,NW,25,ND]
            xw = xw.transpose(4, 3, 1, 2, 0).reshape(NP, RG * NW * C)
            maps.append({
                "feat": fs.reshape(2 * C, RG * 130).astype(bf),
                "xwin": xw.astype(bf),
                "womr": womr, "w2r": w2r,
                "jkt": jkt_t.astype(np.float16),
                "bias": np.asarray(bias, np.float32).reshape(O, 1),
                "bom": np.asarray(b_om, np.float32).reshape(27, 1),
                "dbias": dbias,
                "mxlb": mxlb,
                "one125": np.ones((NP, 1), np.float32),
            })
    return maps


def kernel(input_feat, inter, weight, bias, w_om, b_om):
    if "nc" not in _cache:
        _cache["nc"] = build_bass()
    nc = _cache["nc"]
    maps = host_prep(np.asarray(input_feat, np.float32),
                     np.asarray(inter, np.float32),
                     np.asarray(weight, np.float32),
                     np.asarray(bias, np.float32),
                     np.asarray(w_om, np.float32),
                     np.asarray(b_om, np.float32))
    res = run_bass_kernel_spmd(nc, maps, list(range(NCORES)))
    out = np.zeros((B, O, H, W), np.float32)
    for ci in range(NCORES):
        b, half = ci // 2, ci % 2
        out[b, :, half * HH:(half + 1) * HH] = \
            res.results[ci]["out"].astype(np.float32).reshape(O, HH, W)
    return out
